# revision 15
# baseline (speedup 1.0000x reference)
"""HMM forward-algorithm Trainium2 Bass kernel for nn_HMMCell_26929444946010.

kernel(**inputs) takes FULL unsharded inputs, shards the 256 independent HMM
units across 8 NeuronCores (32 units/core), runs a Bass/Tile kernel per core,
and gathers the FULL [B, T, U] cumulative log-likelihood output.

Per-core algorithm (form-1 layout, states on partitions):
  - 32 units packed as 16 block-diagonal pairs [128=(uh,i), ...].
  - Unnormalized forward recursion fwd~_t = Ē_t ⊙ (A^T fwd~_{t-1}) with
    Ē = 2·B·x_t (doubling keeps the mass drift centered), run in bf16.
  - Per-step transition: one matmul per pair (A-pair stationary [128,128],
    moving fwd~ [128,64]).
  - Emissions: per 8-step block, one matmul per pair (2B-pair stationary
    [8,128], moving replicated-x block [8,512]) -> PSUM, evacuated to SBUF
    bf16 by ScalarE.
  - fwd~ = Ē ⊙ R~ on VectorE, one op per 8-pair group.
  - Mass (= Σ_i fwd~, the per-step likelihood increment): one matmul per
    group per step with a one-hot ones-stationary that lands step t's mass
    at PSUM partitions {2t, 2t+1}, accumulated into a persistent PSUM bank
    over the 64-step body; one batched Ln (scale=2^-(t+1) per partition)
    converts the whole bank at body end, then one contiguous DMA out.
  - Every 64 steps fwd~ is renormalized by the power-of-2 truncation of its
    mass (exponent shift, exact in bf16); divisors are output so the host
    adds the cross-block carry.
"""

import functools
import numpy as np

UNITS, N, S = 256, 64, 4
BATCH, T = 64, 1024
NCORES = 8
UPC = UNITS // NCORES        # 32 units per core
NPAIR = UPC // 2             # 16 pairs
GRP = 8                      # pairs per group
TB = 8                       # steps per emission block
TR = 64                      # steps per body (renorm period)
NBODY = T // TR              # 16
NBLK_PER_BODY = TR // TB     # 8
GW = GRP * BATCH             # 512, free width of one group
LN2 = float(np.log(2.0))


def _softmax(x, axis=-1):
    x = x - np.max(x, axis=axis, keepdims=True)
    e = np.exp(x)
    return e / np.sum(e, axis=axis, keepdims=True)


def _configure(t):
    """Test hook: rebuild module constants for a shorter sequence length."""
    global T, NBODY
    assert t % TR == 0
    T, NBODY = t, t // TR
    _program.cache_clear()


# --------------------------------------------------------------------------
# device program
# --------------------------------------------------------------------------

@functools.lru_cache(maxsize=1)
def _program():
    import concourse.bass as bass
    import concourse.bacc as bacc
    import concourse.tile as tile
    import concourse.mybir as mybir

    f32 = mybir.dt.float32
    bf16 = mybir.dt.bfloat16
    i32 = mybir.dt.int32
    MUL = mybir.AluOpType.mult
    AND = mybir.AluOpType.bitwise_and
    ADD = mybir.AluOpType.add
    LN = mybir.ActivationFunctionType.Ln

    nc = bacc.Bacc("TRN2", target_bir_lowering=False, debug=False,
                   enable_asserts=False, num_devices=NCORES)

    # DRAM tensors (per-core data supplied via in_maps)
    a_d = nc.dram_tensor("a_sb", [128, NPAIR * 128], bf16, kind="ExternalInput").ap()
    b_d = nc.dram_tensor("b_sb", [8, NPAIR * 128], bf16, kind="ExternalInput").ap()
    xx_d = nc.dram_tensor("xx8", [T // TB, 8, TB * BATCH], bf16, kind="ExternalInput").ap()
    ost_d = nc.dram_tensor("onestep", [128, TR * 128], bf16, kind="ExternalInput").ap()
    oneblk_d = nc.dram_tensor("onesblk", [2, 128], bf16, kind="ExternalInput").ap()
    icol_d = nc.dram_tensor("icol", [128, NPAIR], f32, kind="ExternalInput").ap()
    p2_d = nc.dram_tensor("lncol", [128, 1], f32, kind="ExternalInput").ap()
    q_d = nc.dram_tensor("q_out", [NBODY * 128, 2 * GW], f32, kind="ExternalOutput").ap()
    d_d = nc.dram_tensor("d_out", [NBODY, 2, 2 * GW], f32, kind="ExternalOutput").ap()

    with tile.TileContext(nc) as tc:
        with (
            tc.tile_pool(name="const", bufs=1) as cpool,
            tc.tile_pool(name="state", bufs=1) as spool,
            tc.tile_pool(name="esb", bufs=4) as epool,
            tc.tile_pool(name="xxs", bufs=3) as xpool,
            tc.tile_pool(name="rr", bufs=2, space="PSUM") as rrpool,
            tc.tile_pool(name="eps", bufs=2, space="PSUM") as eppool,
            tc.tile_pool(name="macc", bufs=1, space="PSUM") as mpool,
            tc.tile_pool(name="dbcp", bufs=2, space="PSUM") as dpool,
        ):
            # constants
            a_sb = cpool.tile([128, NPAIR * 128], bf16, name="a_sbuf")
            b_sb = cpool.tile([8, NPAIR * 128], bf16, name="b_sbuf")
            ost = cpool.tile([128, TR * 128], bf16, name="onestep_sb")
            oneblk = cpool.tile([2, 128], bf16, name="onesblk_sb")
            icol = cpool.tile([128, NPAIR], f32, name="icol_sb")
            p2col = cpool.tile([128, 1], f32, name="lncol_sb")
            nc.sync.dma_start(a_sb[:, :], a_d[:, :])
            nc.sync.dma_start(b_sb[:, :], b_d[:, :])
            nc.sync.dma_start(ost[:, :], ost_d[:, :])
            nc.sync.dma_start(oneblk[:, :], oneblk_d[:, :])
            nc.sync.dma_start(icol[:, :], icol_d[:, :])
            nc.sync.dma_start(p2col[:, :], p2_d[:, :])

            # persistent state: fwd~ ping-pong per group
            fwd = [[spool.tile([128, GW], bf16, name=f"fwd_g{g}_p{par}")
                    for par in range(2)] for g in range(2)]
            llstage = spool.tile([128, 2 * GW], f32, name="ll_stage")
            dstage = spool.tile([128, 2 * GW], f32, name="d_stage")
            dreci = spool.tile([128, 2 * GW], f32, name="d_recip_f32")
            dstage_bf = spool.tile([128, 2 * GW], bf16, name="d_stage_bf")
            drec = spool.tile([2, 2 * GW], bf16, name="d_recip")

            def emit_estage(blk_expr):
                """Emission block: DMA x-block, 16 E-matmuls, 16 evacs."""
                xx = xpool.tile([8, TB * BATCH], bf16, name="xx_stage", tag="xx")
                nc.sync.dma_start(xx[:, :], xx_d[bass.ds(blk_expr, 1), :, :])
                etiles = []
                for g in range(2):
                    esb = epool.tile([128, GRP * TB * BATCH], bf16,
                                     name=f"e_sb_g{g}", tag=f"esb{g}")
                    for p8 in range(GRP):
                        p = g * GRP + p8
                        eps = eppool.tile([128, TB * BATCH], f32,
                                          name="e_ps", tag="eps")
                        nc.tensor.matmul(
                            eps[:, :],
                            b_sb[:, p * 128:(p + 1) * 128],
                            xx[:, :],
                            start=True, stop=True)
                        nc.scalar.copy(
                            esb[:, p8 * (TB * BATCH):(p8 + 1) * (TB * BATCH)],
                            eps[:, :])
                    etiles.append(esb)
                return etiles

            def eview(etile, tq):
                """[128, (8 pair, 64 b)] view of an emission tile at step tq."""
                r = etile[:, :].rearrange("p (pr tb) -> p pr tb", pr=GRP)
                return r[:, :, tq * BATCH:(tq + 1) * BATCH]

            def emit_step(t, etiles, maccs, first_init=False):
                """One recursion step t (reads fwd[(t+1)%2], writes fwd[t%2])."""
                tq = t % TB
                wpar, rpar = t % 2, (t + 1) % 2
                for g in range(2):
                    if first_init:
                        # fwd~_0 = Ē_0 ⊙ icol  (per-pair per-partition scalar)
                        for p8 in range(GRP):
                            p = g * GRP + p8
                            nc.vector.tensor_scalar_mul(
                                fwd[g][wpar][:, p8 * BATCH:(p8 + 1) * BATCH],
                                etiles[g][:, p8 * (TB * BATCH):p8 * (TB * BATCH) + BATCH],
                                icol[:, p:p + 1])
                    else:
                        rr = rrpool.tile([128, GW], f32, name="rr", tag="rr")
                        for p8 in range(GRP):
                            p = g * GRP + p8
                            nc.tensor.matmul(
                                rr[:, p8 * BATCH:(p8 + 1) * BATCH],
                                a_sb[:, p * 128:(p + 1) * 128],
                                fwd[g][rpar][:, p8 * BATCH:(p8 + 1) * BATCH],
                                start=True, stop=True)
                        nc.vector.tensor_tensor(
                            fwd[g][wpar][:, :].rearrange("p (pr b) -> p pr b", pr=GRP),
                            rr[:, :].rearrange("p (pr b) -> p pr b", pr=GRP),
                            eview(etiles[g], tq),
                            MUL)
                    # mass: one accumulating matmul; one-hot cols land step t's
                    # mass at PSUM partitions {2t, 2t+1}
                    nc.tensor.matmul(
                        maccs[g][:, :],
                        ost[:, t * 128:(t + 1) * 128],
                        fwd[g][wpar][:, :],
                        start=(t == 0), stop=(t == TR - 1),
                        skip_group_check=True)

            def emit_body(i_expr, first):
                maccs = [mpool.tile([128, GW], f32, name=f"macc{g}", tag=f"macc{g}")
                         for g in range(2)]
                for kb in range(NBLK_PER_BODY):
                    if first:
                        blk = kb
                    else:
                        blk = i_expr * NBLK_PER_BODY + kb
                    etiles = emit_estage(blk)
                    for tq in range(TB):
                        t = kb * TB + tq
                        emit_step(t, etiles, maccs,
                                  first_init=(first and t == 0))
                # ---- renorm: pow2-truncate mass_63, recip, broadcast, scale
                for g in range(2):
                    sl = slice(g * GW, (g + 1) * GW)
                    nc.vector.tensor_scalar(
                        dstage[96:128, sl].bitcast(i32),
                        maccs[g][96:128, :].bitcast(i32),
                        -8388608,  # 0xFF800000
                        None, AND)
                nc.vector.tensor_scalar(
                    dreci[96:128, :].bitcast(i32),
                    dstage[96:128, :].bitcast(i32),
                    -1, 2130706432,  # (254<<23)
                    MUL, ADD)
                nc.vector.tensor_copy(dstage_bf[96:128, :], dreci[96:128, :])
                # move recip rows 126:128 -> drec rows 0:2 (partition remap)
                nc.sync.dma_start(drec[0:2, :], dstage_bf[126:128, :])
                for g in range(2):
                    dbc = dpool.tile([128, GW], f32, name="dbc", tag="dbc")
                    for p8 in range(GRP):
                        p = g * GRP + p8
                        nc.tensor.matmul(
                            dbc[:, p8 * BATCH:(p8 + 1) * BATCH],
                            oneblk[:, :],
                            drec[0:2, p * BATCH:(p + 1) * BATCH],
                            start=True, stop=True)
                    nc.vector.tensor_tensor(
                        fwd[g][1][:, :], fwd[g][1][:, :], dbc[:, :], MUL)
                # ---- batched ll: ll = Ln(mass) - (t+1)·ln2 per partition row
                # (Ln is applied to the raw mass: the ACT Ln LUT clamps tiny
                # inputs, so the 2^-(t+1) step correction is added after.)
                for g in range(2):
                    nc.scalar.activation(
                        llstage[:, g * GW:(g + 1) * GW], maccs[g][:, :], LN)
                nc.vector.tensor_scalar(
                    llstage[:, :], llstage[:, :], p2col[:, 0:1], None, ADD)
                if first:
                    nc.sync.dma_start(q_d[0:128, :], llstage[:, :])
                    nc.sync.dma_start(d_d[0:1, :, :], dstage[126:128, :])
                else:
                    nc.sync.dma_start(
                        q_d[bass.ds(i_expr * 128, 128), :], llstage[:, :])
                    nc.sync.dma_start(
                        d_d[bass.ds(i_expr, 1), :, :], dstage[126:128, :])

            emit_body(0, True)
            if NBODY > 1:
                with tc.For_i(1, NBODY) as i:
                    emit_body(i, False)

    nc.compile()
    return nc


# --------------------------------------------------------------------------
# host side
# --------------------------------------------------------------------------

def _host_prep(inputs, transition_kernel, emission_kernel, init_kernel):
    x = np.asarray(inputs, dtype=np.float32)            # [B, T, S]
    A = _softmax(np.asarray(transition_kernel, np.float32))  # [U, N, N]
    Bm = _softmax(np.asarray(emission_kernel, np.float32))   # [U, N, S]
    I = _softmax(np.asarray(init_kernel, np.float32))        # [U, N]

    # x block tensor: xx8[blk, uh*4+c, tq*B + b] = x[b, blk*TB+tq, c]
    xt = x.transpose(1, 2, 0)                            # [T, S, B]
    xt = xt.reshape(T // TB, TB, S, BATCH)               # [blk, tq, c, b]
    half = xt.transpose(0, 2, 1, 3).reshape(T // TB, S, TB * BATCH)
    xx8 = np.concatenate([half, half], axis=1)           # [blk, 8, TB*B]

    # per-step mass stationaries: onestep[:, t*128 + m]
    onestep = np.zeros((128, TR * 128), np.float32)
    for t in range(TR):
        onestep[0:64, t * 128 + 2 * t] = 1.0
        onestep[64:128, t * 128 + 2 * t + 1] = 1.0
    onesblk = np.zeros((2, 128), np.float32)
    onesblk[0, 0:64] = 1.0
    onesblk[1, 64:128] = 1.0
    # per mass partition (2t+uh): -(t+1)·ln2, added after the Ln
    lncol = np.zeros((128, 1), np.float64)
    for t in range(TR):
        lncol[2 * t, 0] = lncol[2 * t + 1, 0] = -(t + 1) * LN2
    lncol = lncol.astype(np.float32)

    def to_bf16(a):
        import ml_dtypes
        return np.asarray(a, np.float32).astype(ml_dtypes.bfloat16)

    xx8_bf = to_bf16(xx8)
    onestep_bf = to_bf16(onestep)
    onesblk_bf = to_bf16(onesblk)

    in_maps = []
    for c in range(NCORES):
        u0 = c * UPC
        a_sb = np.zeros((128, NPAIR * 128), np.float32)
        b_sb = np.zeros((8, NPAIR * 128), np.float32)
        icol = np.zeros((128, NPAIR), np.float32)
        for p in range(NPAIR):
            ua, ub = u0 + 2 * p, u0 + 2 * p + 1
            a_sb[0:64, p * 128:p * 128 + 64] = A[ua]          # lhsT[i, j]
            a_sb[64:128, p * 128 + 64:p * 128 + 128] = A[ub]
            for cc in range(S):
                b_sb[cc, p * 128:p * 128 + 64] = 2.0 * Bm[ua, :, cc]
                b_sb[4 + cc, p * 128 + 64:p * 128 + 128] = 2.0 * Bm[ub, :, cc]
            icol[0:64, p] = I[ua]
            icol[64:128, p] = I[ub]
        in_maps.append({
            "a_sb": to_bf16(a_sb),
            "b_sb": to_bf16(b_sb),
            "xx8": xx8_bf,
            "onestep": onestep_bf,
            "onesblk": onesblk_bf,
            "icol": icol,
            "lncol": lncol,
        })
    return in_maps


def _host_post(results):
    """results: per-core dicts with q_out [NBODY*128, 2*GW], d_out [NBODY,2,2*GW]."""
    out = np.empty((BATCH, T, UNITS), np.float32)
    for c in range(NCORES):
        q = np.asarray(results[c]["q_out"], np.float64)
        d = np.asarray(results[c]["d_out"], np.float64)
        # q rows: (body, tloc, uh); cols: (g, pr, b)
        q = q.reshape(NBODY, TR, 2, 2, GRP, BATCH)       # k, tl, uh, g, pr, b
        d = d.reshape(NBODY, 2, 2, GRP, BATCH)           # k, uh, g, pr, b
        ln_d = np.log(d) - TR * LN2
        carry = np.cumsum(ln_d, axis=0)
        carry = np.concatenate(
            [np.zeros((1,) + carry.shape[1:]), carry[:-1]], axis=0)
        ll = q + carry[:, None, :, :, :, :]              # k, tl, uh, g, pr, b
        # u_local = g*16 + pr*2 + uh
        ll = ll.reshape(T, 2, 2, GRP, BATCH)             # t, uh, g, pr, b
        ll = ll.transpose(4, 0, 2, 3, 1).reshape(BATCH, T, UPC)
        out[:, :, c * UPC:(c + 1) * UPC] = ll.astype(np.float32)
    return out


def _host_first_steps(x, A, Bm, I, k=4):
    """Exact ll for the first k steps (the small-|ll| region) in f32/f64."""
    x = x[:, :k, :].astype(np.float64)
    A64, B64, I64 = A.astype(np.float64), Bm.astype(np.float64), I.astype(np.float64)
    Bb = x.shape[0]
    alpha = np.zeros((Bb, UNITS, N))
    ll = np.zeros((Bb, UNITS))
    out = np.empty((Bb, k, UNITS))
    for t in range(k):
        if t == 0:
            R = np.broadcast_to(I64[None], (Bb, UNITS, N))
        else:
            R = np.einsum("uij,bui->buj", A64, alpha, optimize=True)
        E = np.einsum("unc,bc->bun", B64, x[:, t, :], optimize=True)
        fwd = E * R
        Ss = fwd.sum(-1)
        ll = ll + np.log(Ss)
        alpha = fwd / Ss[..., None]
        out[:, t, :] = ll
    return out.astype(np.float32)


def _run(in_maps, trace=False, **kw):
    from concourse import bass_utils
    nc = _program()
    return bass_utils.run_bass_kernel_spmd(
        nc, in_maps, core_ids=list(range(NCORES)), trace=trace, **kw)


def kernel(inputs, transition_kernel, emission_kernel, init_kernel):
    x = np.asarray(inputs, dtype=np.float32)
    A = _softmax(np.asarray(transition_kernel, np.float32))
    Bm = _softmax(np.asarray(emission_kernel, np.float32))
    I = _softmax(np.asarray(init_kernel, np.float32))
    in_maps = _host_prep(inputs, transition_kernel, emission_kernel, init_kernel)
    res = _run(in_maps)
    out = _host_post(res.results)
    out[:, :4, :] = _host_first_steps(x, A, Bm, I, k=4)
    return out
